# revision 4
# baseline (speedup 1.0000x reference)
"""Bass/Tile Trainium2 kernel for additive (Bahdanau/'cat') attention.

Problem (per batch b):
  A[i,d]      = sum_a context[i,a] * attn_w[a,d] + attn_b[d]
  O[o,d]      = sum_e output[o,e]  * dec_w[e,d]  + dec_b[d]
  scores[o,i] = sum_d query_w[d] * tanh(A[i,d] + O[o,d])   (+query_b: softmax-invariant)
  attn        = softmax_i(scores)
  mix[o,a]    = sum_i attn[o,i] * context[i,a]
  out[o,d]    = tanh([mix | output] @ out_w + out_b)

Sharding: pure data-parallel over batch, B=8 -> one batch per NeuronCore,
weights broadcast, no collectives.

Per-core layout choices:
  * everything that feeds the big tanh is kept transposed with d on
    partitions: A^T [d, i], O^T [d, o].  The broadcast add A^T + O^T[:,o]
    is a DVE tensor_scalar_add (per-partition scalar), batched 8 o's per
    ACT tanh instruction (free dim 4096) to amortize ACT overhead.
  * the q-reduction over d is a PE matmul with lhsT = q[dchunk] [128,1],
    rhs = tanh tile [128, 512], PSUM-accumulated over the 4 d-chunks.
  * softmax uses ACT Exp with fused accum_out for the denominator.
"""

import numpy as np

import concourse.bass as bass
import concourse.tile as tile
import concourse.bass_utils as bass_utils
from concourse import bacc, mybir
from concourse.masks import make_identity

B, OUT_LEN, IN_LEN, DEC, ATTN = 8, 64, 512, 512, 512
P = 128
F32 = mybir.dt.float32
BF16 = mybir.dt.bfloat16
AF = mybir.ActivationFunctionType

G = 8                     # o's per tanh group
NG = OUT_LEN // G         # 8 groups
DC = DEC // P             # 4 d-chunks
AC = ATTN // P            # 4 a-chunks
IC = IN_LEN // P          # 4 i-chunks
EC = DEC // P             # 4 e-chunks (decoder feature)
CC = (ATTN + DEC) // P    # 8 combined chunks

N_CORES = 8


def _build_body(tc):
    nc = tc.nc

    # ---- DRAM I/O (per-core shard shapes) ----
    output_d = nc.dram_tensor("output", [OUT_LEN, DEC], F32, kind="ExternalInput").ap()
    context_d = nc.dram_tensor("context", [IN_LEN, ATTN], F32, kind="ExternalInput").ap()
    dec_w_d = nc.dram_tensor("dec_w", [DEC, DEC], F32, kind="ExternalInput").ap()
    dec_b_d = nc.dram_tensor("dec_b", [DEC, 1], F32, kind="ExternalInput").ap()
    attn_w_d = nc.dram_tensor("attn_w", [ATTN, DEC], F32, kind="ExternalInput").ap()
    attn_b_d = nc.dram_tensor("attn_b", [ATTN, 1], F32, kind="ExternalInput").ap()
    query_w_d = nc.dram_tensor("query_w", [DEC, 1], F32, kind="ExternalInput").ap()
    out_w_d = nc.dram_tensor("out_w", [ATTN + DEC, DEC], F32, kind="ExternalInput").ap()
    out_b_d = nc.dram_tensor("out_b", [DEC, 1], F32, kind="ExternalInput").ap()
    out_d = nc.dram_tensor("out", [OUT_LEN, DEC], F32, kind="ExternalOutput").ap()
    attn_d = nc.dram_tensor("attn", [OUT_LEN, IN_LEN], F32, kind="ExternalOutput").ap()

    from contextlib import ExitStack

    with ExitStack() as ctx:
        const = ctx.enter_context(tc.tile_pool(name="const", bufs=1))
        statics = ctx.enter_context(tc.tile_pool(name="statics", bufs=1))
        epool = ctx.enter_context(tc.tile_pool(name="epool", bufs=3))
        fpool = ctx.enter_context(tc.tile_pool(name="fpool", bufs=2))
        spool = ctx.enter_context(tc.tile_pool(name="spool", bufs=2))
        psum = ctx.enter_context(tc.tile_pool(name="psum", bufs=2, space="PSUM"))

        # ---------------- constants / small inputs ----------------
        ident = const.tile([P, P], F32)
        make_identity(nc, ident)

        attn_bias = const.tile([P, DC], F32)
        dec_bias = const.tile([P, DC], F32)
        out_bias = const.tile([P, DC], F32)
        qw_f = const.tile([P, DC], F32)
        qw_bf = const.tile([P, DC], BF16)
        for dc in range(DC):
            nc.sync.dma_start(attn_bias[:, dc : dc + 1], attn_b_d[dc * P : (dc + 1) * P, :])
            nc.sync.dma_start(dec_bias[:, dc : dc + 1], dec_b_d[dc * P : (dc + 1) * P, :])
            nc.sync.dma_start(out_bias[:, dc : dc + 1], out_b_d[dc * P : (dc + 1) * P, :])
            nc.sync.dma_start(qw_f[:, dc : dc + 1], query_w_d[dc * P : (dc + 1) * P, :])
        nc.vector.tensor_copy(qw_bf[:], qw_f[:])

        # ---------------- big input DMAs ----------------
        ctx_sb = statics.tile([P, IC, ATTN], F32)      # [i%, ic, a]
        attn_w_sb = statics.tile([P, AC, DEC], F32)    # [a%, ac, d]
        dec_w_sb = statics.tile([P, EC, DEC], F32)     # [e%, ec, d]
        output_sb = statics.tile([OUT_LEN, DEC], F32)  # [o, e]
        out_w_sb = statics.tile([P, CC, DEC], F32)     # [c%, cc, d]
        for ic in range(IC):
            nc.sync.dma_start(ctx_sb[:, ic, :], context_d[ic * P : (ic + 1) * P, :])
        for ac in range(AC):
            nc.sync.dma_start(attn_w_sb[:, ac, :], attn_w_d[ac * P : (ac + 1) * P, :])
        nc.sync.dma_start(output_sb[:], output_d[:])
        for ec in range(EC):
            nc.sync.dma_start(dec_w_sb[:, ec, :], dec_w_d[ec * P : (ec + 1) * P, :])

        # ---------------- transposes: context^T, output^T ----------------
        ctxT_sb = statics.tile([P, AC, IN_LEN], F32)   # [a%, ac, i]
        for ic in range(IC):
            for ac in range(AC):
                pt = psum.tile([P, P], F32, tag="tp")
                nc.tensor.transpose(pt[:], ctx_sb[:, ic, ac * P : (ac + 1) * P], ident[:])
                nc.vector.tensor_copy(ctxT_sb[:, ac, ic * P : (ic + 1) * P], pt[:])

        # combined^T [c%, cc, o]: chunks 0..3 = mix^T (filled later), 4..7 = output^T
        combT_sb = statics.tile([P, CC, OUT_LEN], F32)
        for ec in range(EC):
            pt = psum.tile([P, OUT_LEN], F32, tag="tp")
            nc.tensor.transpose(
                pt[:], output_sb[0:OUT_LEN, ec * P : (ec + 1) * P], ident[0:OUT_LEN, 0:OUT_LEN]
            )
            nc.vector.tensor_copy(combT_sb[:, EC + ec, :], pt[:])

        # ---------------- A^T = attn_w^T @ context^T (+attn_b), bf16 ----------------
        ATb = statics.tile([P, DC, IN_LEN], BF16)      # [d%, dc, i]
        for dc in range(DC):
            pa = psum.tile([P, IN_LEN], F32, tag="mm")
            for ac in range(AC):
                nc.tensor.matmul(
                    pa[:],
                    attn_w_sb[:, ac, dc * P : (dc + 1) * P],
                    ctxT_sb[:, ac, :],
                    start=(ac == 0),
                    stop=(ac == AC - 1),
                )
            nc.vector.tensor_scalar_add(ATb[:, dc, :], pa[:], attn_bias[:, dc : dc + 1])

        # ---------------- O^T = dec_w^T @ output^T (+dec_b), bf16 ----------------
        OTb = statics.tile([P, DC, OUT_LEN], F32)      # [d%, dc, o]
        for dc in range(DC):
            po = psum.tile([P, OUT_LEN], F32, tag="sm")
            for ec in range(EC):
                nc.tensor.matmul(
                    po[:],
                    dec_w_sb[:, ec, dc * P : (dc + 1) * P],
                    combT_sb[:, EC + ec, :],
                    start=(ec == 0),
                    stop=(ec == EC - 1),
                )
            nc.vector.tensor_scalar_add(OTb[:, dc, :], po[:], dec_bias[:, dc : dc + 1])

        # ---------------- main loop: tanh + q-reduction ----------------
        scores_sb = statics.tile([OUT_LEN, IN_LEN], F32)
        for og in range(NG):
            Fg = fpool.tile([P, DC, G, IN_LEN], BF16)
            for dc in range(DC):
                E = epool.tile([P, G, IN_LEN], BF16)
                for j in range(G):
                    o = og * G + j
                    nc.vector.tensor_scalar_add(
                        E[:, j, :], ATb[:, dc, :], OTb[:, dc, o : o + 1]
                    )
                nc.scalar.activation(Fg[:, dc], E[:], AF.Tanh)
            # stage the group's 8 score rows on partition 0, then one
            # SBUF->SBUF DMA scatters them to rows og*G..og*G+G-1
            # (compute engines can only write partition starts {0,32,64,96}).
            stage = spool.tile([1, G, IN_LEN], F32)
            for j in range(G):
                ps = psum.tile([1, IN_LEN], F32, tag="sc")
                for dc in range(DC):
                    nc.tensor.matmul(
                        ps[:],
                        qw_bf[:, dc : dc + 1],
                        Fg[:, dc, j],
                        start=(dc == 0),
                        stop=(dc == DC - 1),
                    )
                nc.vector.tensor_copy(stage[0:1, j, :], ps[:])
            nc.sync.dma_start(scores_sb[og * G : (og + 1) * G, :], stage[0:1, :, :])

        # ---------------- softmax over i ----------------
        exp_sb = statics.tile([OUT_LEN, IN_LEN], F32)
        sums = statics.tile([OUT_LEN, 1], F32)
        recip = statics.tile([OUT_LEN, 1], F32)
        attn_sb = statics.tile([OUT_LEN, IN_LEN], F32)
        nc.scalar.activation(exp_sb[:], scores_sb[:], AF.Exp, accum_out=sums[:])
        nc.vector.reciprocal(recip[:], sums[:])
        nc.vector.tensor_scalar_mul(attn_sb[:], exp_sb[:], recip[:])
        nc.sync.dma_start(attn_d[:], attn_sb[:])

        # ---------------- attn^T, mix^T ----------------
        attnT_sb = statics.tile([P, IC, OUT_LEN], F32)  # [i%, ic, o]
        for ic in range(IC):
            pt = psum.tile([P, OUT_LEN], F32, tag="sm")
            nc.tensor.transpose(
                pt[:], attn_sb[0:OUT_LEN, ic * P : (ic + 1) * P], ident[0:OUT_LEN, 0:OUT_LEN]
            )
            nc.vector.tensor_copy(attnT_sb[:, ic, :], pt[:])
        for ac in range(AC):
            pm = psum.tile([P, OUT_LEN], F32, tag="sm")
            for ic in range(IC):
                nc.tensor.matmul(
                    pm[:],
                    ctx_sb[:, ic, ac * P : (ac + 1) * P],
                    attnT_sb[:, ic, :],
                    start=(ic == 0),
                    stop=(ic == IC - 1),
                )
            nc.vector.tensor_copy(combT_sb[:, ac, :], pm[:])

        # ---------------- out^T = tanh(out_w^T @ combined^T + out_b) ----------------
        for cc in range(CC):
            nc.sync.dma_start(out_w_sb[:, cc, :], out_w_d[cc * P : (cc + 1) * P, :])
        outT_sb = statics.tile([P, DC, OUT_LEN], F32)
        for dc in range(DC):
            pf = psum.tile([P, OUT_LEN], F32, tag="sm")
            for cc in range(CC):
                nc.tensor.matmul(
                    pf[:],
                    out_w_sb[:, cc, dc * P : (dc + 1) * P],
                    combT_sb[:, cc, :],
                    start=(cc == 0),
                    stop=(cc == CC - 1),
                )
            nc.scalar.activation(
                outT_sb[:, dc, :], pf[:], AF.Tanh, bias=out_bias[:, dc : dc + 1]
            )

        # ---------------- un-transpose + store ----------------
        out_sb = statics.tile([OUT_LEN, DEC], F32)
        for dc in range(DC):
            pt = psum.tile([OUT_LEN, P], F32, tag="tp")
            nc.tensor.transpose(pt[:], outT_sb[:, dc, :], ident[:])
            nc.vector.tensor_copy(out_sb[0:OUT_LEN, dc * P : (dc + 1) * P], pt[:])
        nc.sync.dma_start(out_d[:], out_sb[:])


_CACHE = {}


def build_nc():
    if "nc" in _CACHE:
        return _CACHE["nc"]
    nc = bacc.Bacc(
        "TRN2",
        target_bir_lowering=False,
        debug=False,
        num_devices=N_CORES,
    )
    with tile.TileContext(nc) as tc:
        _build_body(tc)
    nc.compile()
    _CACHE["nc"] = nc
    return nc


def kernel(**inputs):
    nc = build_nc()

    f = lambda k: np.ascontiguousarray(np.asarray(inputs[k], dtype=np.float32))
    output = f("output")
    context = f("context")
    shared = {
        "dec_w": f("dec_w"),
        "dec_b": f("dec_b").reshape(DEC, 1),
        "attn_w": f("attn_w"),
        "attn_b": f("attn_b").reshape(ATTN, 1),
        "query_w": f("query_w").reshape(DEC, 1),
        "out_w": f("out_w"),
        "out_b": f("out_b").reshape(DEC, 1),
    }
    in_maps = []
    for b in range(N_CORES):
        m = dict(shared)
        m["output"] = np.ascontiguousarray(output[b])
        m["context"] = np.ascontiguousarray(context[b])
        in_maps.append(m)

    res = bass_utils.run_bass_kernel_spmd(nc, in_maps, core_ids=list(range(N_CORES)))
    _CACHE["last_results"] = res
    out = np.stack([res.results[b]["out"] for b in range(N_CORES)])
    attn = np.stack([res.results[b]["attn"] for b in range(N_CORES)])
    return out, attn


# revision 10
# speedup vs baseline: 1.0162x; 1.0162x over previous
"""Bass/Tile Trainium2 kernel for additive (Bahdanau/'cat') attention.

Problem (per batch b):
  A[i,d]      = sum_a context[i,a] * attn_w[a,d] + attn_b[d]
  O[o,d]      = sum_e output[o,e]  * dec_w[e,d]  + dec_b[d]
  scores[o,i] = sum_d query_w[d] * tanh(A[i,d] + O[o,d])   (+query_b: softmax-invariant)
  attn        = softmax_i(scores)
  mix[o,a]    = sum_i attn[o,i] * context[i,a]
  out[o,d]    = tanh([mix | output] @ out_w + out_b)

Sharding: pure data-parallel over batch, B=8 -> one batch per NeuronCore,
weights broadcast, no collectives.

Per-core structure:
  * A^T [d,i] and O^T [d,o] kept with d on partitions so the broadcast add
    A^T + O^T[:,o] is a DVE tensor_scalar (per-partition scalar), in bf16.
  * tanh batched 8 o's per ACT instruction (free dim 4096); d-chunk-outer
    so the PE gets matmul work after every ACT chunk (keeps HAM warm).
  * q-reduction over d on the PE with zero-padded stationary operand:
    lhsT QZ[:,dc,j] is [128,8] holding query_w in column j -> all 32
    matmuls of a group accumulate into ONE [8,512] PSUM bank; one cheap
    8-row DVE copy + SBUF->SBUF DMA scatters rows into scores.
  * softmax/mix/out epilogue computed in row-halves (0:32 during groups
    4..7, 32:64 at the end) to shorten the serial tail.
"""

import numpy as np

import concourse.bass as bass
import concourse.tile as tile
import concourse.bass_utils as bass_utils
from concourse import bacc, mybir
from concourse.masks import make_identity

B, OUT_LEN, IN_LEN, DEC, ATTN = 8, 64, 512, 512, 512
P = 128
F32 = mybir.dt.float32
BF16 = mybir.dt.bfloat16
AF = mybir.ActivationFunctionType

G = 8                     # o's per tanh group
NG = OUT_LEN // G         # 8 groups
DC = DEC // P             # 4 d-chunks
AC = ATTN // P            # 4 a-chunks
IC = IN_LEN // P          # 4 i-chunks
EC = DEC // P             # 4 e-chunks (decoder feature)
CC = (ATTN + DEC) // P    # 8 combined chunks
H = OUT_LEN // 2          # row half

N_CORES = 8


def _epilogue_half(nc, h, ident, scores_sb, exp_sb, sums, recip, attn_sb,
                   attnT_bf, ctx_bf, combT_sb, out_w_sb, out_bias, outT_sb,
                   out_sb, psum, attn_d, out_d):
    """softmax + mix + output projection for rows h*32..h*32+31."""
    r0 = h * H
    sl = slice(r0, r0 + H)
    nc.scalar.activation(exp_sb[sl, :], scores_sb[sl, :], AF.Exp, accum_out=sums[sl, :])
    nc.vector.reciprocal(recip[sl, :], sums[sl, :])
    nc.vector.tensor_scalar_mul(attn_sb[sl, :], exp_sb[sl, :], recip[sl, :])
    nc.sync.dma_start(attn_d[sl, :], attn_sb[sl, :])

    # attn^T (converted to bf16 in the PSUM evac) for this half
    for ic in range(IC):
        pt = psum.tile([P, H], F32, tag="tp", name=f"pt_at_{h}_{ic}")
        nc.tensor.transpose(
            pt[:], attn_sb[sl, ic * P : (ic + 1) * P], ident[sl, r0 : r0 + H]
        )
        nc.vector.tensor_copy(attnT_bf[:, ic, sl], pt[:])

    # mix^T -> combined chunks 0..3
    for ac in range(AC):
        pm = psum.tile([P, H], F32, tag="sm", name=f"pm_{h}_{ac}")
        for ic in range(IC):
            nc.tensor.matmul(
                pm[:],
                ctx_bf[:, ic, ac * P : (ac + 1) * P],
                attnT_bf[:, ic, sl],
                start=(ic == 0),
                stop=(ic == IC - 1),
            )
        nc.vector.tensor_copy(combT_sb[:, ac, sl], pm[:])

    # out^T = tanh(out_w^T @ combined^T + out_b)
    for dc in range(DC):
        pf = psum.tile([P, H], F32, tag="sm", name=f"pf_{h}_{dc}")
        for cc in range(CC):
            nc.tensor.matmul(
                pf[:],
                out_w_sb[:, cc, dc * P : (dc + 1) * P],
                combT_sb[:, cc, sl],
                start=(cc == 0),
                stop=(cc == CC - 1),
            )
        nc.scalar.activation(
            outT_sb[:, dc, sl], pf[:], AF.Tanh, bias=out_bias[:, dc : dc + 1]
        )

    # un-transpose to [o, d] rows and store
    for dc in range(DC):
        pt = psum.tile([H, P], F32, tag="tp", name=f"pt_o_{h}_{dc}")
        nc.tensor.transpose(pt[:], outT_sb[:, dc, sl], ident[:])
        nc.vector.tensor_copy(out_sb[sl, dc * P : (dc + 1) * P], pt[:])
    nc.sync.dma_start(out_d[sl, :], out_sb[sl, :])


def _build_body(tc):
    nc = tc.nc

    # ---- DRAM I/O (per-core shard shapes) ----
    output_d = nc.dram_tensor("output", [OUT_LEN, DEC], F32, kind="ExternalInput").ap()
    context_d = nc.dram_tensor("context", [IN_LEN, ATTN], F32, kind="ExternalInput").ap()
    dec_w_d = nc.dram_tensor("dec_w", [DEC, DEC], F32, kind="ExternalInput").ap()
    dec_b_d = nc.dram_tensor("dec_b", [DEC, 1], F32, kind="ExternalInput").ap()
    attn_w_d = nc.dram_tensor("attn_w", [ATTN, DEC], F32, kind="ExternalInput").ap()
    attn_b_d = nc.dram_tensor("attn_b", [ATTN, 1], F32, kind="ExternalInput").ap()
    query_w_d = nc.dram_tensor("query_w", [DEC, 1], F32, kind="ExternalInput").ap()
    out_w_d = nc.dram_tensor("out_w", [ATTN + DEC, DEC], F32, kind="ExternalInput").ap()
    out_b_d = nc.dram_tensor("out_b", [DEC, 1], F32, kind="ExternalInput").ap()
    out_d = nc.dram_tensor("out", [OUT_LEN, DEC], F32, kind="ExternalOutput").ap()
    attn_d = nc.dram_tensor("attn", [OUT_LEN, IN_LEN], F32, kind="ExternalOutput").ap()

    from contextlib import ExitStack

    with ExitStack() as ctx:
        const = ctx.enter_context(tc.tile_pool(name="const", bufs=1))
        statics = ctx.enter_context(tc.tile_pool(name="statics", bufs=1))
        epool = ctx.enter_context(tc.tile_pool(name="epool", bufs=3))
        fpool = ctx.enter_context(tc.tile_pool(name="fpool", bufs=2))
        spool = ctx.enter_context(tc.tile_pool(name="spool", bufs=2))
        psum = ctx.enter_context(tc.tile_pool(name="psum", bufs=2, space="PSUM"))

        # ---------------- constants / small inputs ----------------
        ident = const.tile([P, P], F32)
        make_identity(nc, ident)
        ident_bf = const.tile([P, P], BF16)
        nc.vector.tensor_copy(ident_bf[:], ident[:])

        attn_bias = const.tile([P, DC], F32)
        dec_bias = const.tile([P, DC], F32)
        out_bias = const.tile([P, DC], F32)
        qw_f = const.tile([P, DC], F32)
        qw_bf = const.tile([P, DC], BF16)
        for dc in range(DC):
            nc.sync.dma_start(attn_bias[:, dc : dc + 1], attn_b_d[dc * P : (dc + 1) * P, :])
            nc.sync.dma_start(dec_bias[:, dc : dc + 1], dec_b_d[dc * P : (dc + 1) * P, :])
            nc.sync.dma_start(out_bias[:, dc : dc + 1], out_b_d[dc * P : (dc + 1) * P, :])
            nc.sync.dma_start(qw_f[:, dc : dc + 1], query_w_d[dc * P : (dc + 1) * P, :])
        nc.vector.tensor_copy(qw_bf[:], qw_f[:])

        # zero-padded stationary operands: QZ[:, dc, j] is [128, G] with
        # query_w[dc] in column j, zeros elsewhere -> matmul j deposits
        # scores for o_j into PSUM row j, rows != j accumulate zeros.
        QZ = const.tile([P, DC, G, G], BF16)
        nc.vector.memset(QZ[:], 0.0)
        for dc in range(DC):
            for j in range(G):
                nc.vector.tensor_copy(QZ[:, dc, j, j : j + 1], qw_bf[:, dc : dc + 1])

        # ---------------- big input DMAs (split for queue parallelism) ----
        ctx_sb = statics.tile([P, IC, ATTN], F32)      # [i%, ic, a]
        attn_w_sb = statics.tile([P, AC, DEC], F32)    # [a%, ac, d]
        dec_w_sb = statics.tile([P, EC, DEC], F32)     # [e%, ec, d]
        output_sb = statics.tile([OUT_LEN, DEC], F32)  # [o, e]
        out_w_sb = statics.tile([P, CC, DEC], F32)     # [c%, cc, d]
        for ic in range(IC):
            for half in range(2):
                nc.sync.dma_start(
                    ctx_sb[:, ic, half * 256 : (half + 1) * 256],
                    context_d[ic * P : (ic + 1) * P, half * 256 : (half + 1) * 256],
                )
        nc.sync.dma_start(output_sb[:], output_d[:])
        for ec in range(EC):
            for half in range(2):
                nc.sync.dma_start(
                    dec_w_sb[:, ec, half * 256 : (half + 1) * 256],
                    dec_w_d[ec * P : (ec + 1) * P, half * 256 : (half + 1) * 256],
                )
        for ac in range(AC):
            for half in range(2):
                nc.sync.dma_start(
                    attn_w_sb[:, ac, half * 256 : (half + 1) * 256],
                    attn_w_d[ac * P : (ac + 1) * P, half * 256 : (half + 1) * 256],
                )

        # bf16 copies for the attention-score pipeline
        ctx_bf = statics.tile([P, IC, ATTN], BF16)
        attn_w_bf = statics.tile([P, AC, DEC], BF16)
        for ic in range(IC):
            nc.vector.tensor_copy(ctx_bf[:, ic, :], ctx_sb[:, ic, :])
        for ac in range(AC):
            nc.vector.tensor_copy(attn_w_bf[:, ac, :], attn_w_sb[:, ac, :])

        # ---------------- transposes: context^T (bf16), output^T ----------
        ctxT_bf = statics.tile([P, AC, IN_LEN], BF16)  # [a%, ac, i]
        for ic in range(IC):
            for ac in range(AC):
                pt = psum.tile([P, P], BF16, tag="tp", name=f"pt_c_{ic}_{ac}")
                nc.tensor.transpose(pt[:], ctx_bf[:, ic, ac * P : (ac + 1) * P], ident_bf[:])
                nc.vector.tensor_copy(ctxT_bf[:, ac, ic * P : (ic + 1) * P], pt[:])

        # combined^T [c%, cc, o]: chunks 0..3 = mix^T (later), 4..7 = output^T
        combT_sb = statics.tile([P, CC, OUT_LEN], F32)
        for ec in range(EC):
            pt = psum.tile([P, OUT_LEN], F32, tag="tp", name=f"pt_ot_{ec}")
            nc.tensor.transpose(
                pt[:], output_sb[0:OUT_LEN, ec * P : (ec + 1) * P],
                ident[0:OUT_LEN, 0:OUT_LEN],
            )
            nc.vector.tensor_copy(combT_sb[:, EC + ec, :], pt[:])

        # ---------------- A^T (bf16 matmul) and O^T ----------------
        ATb = statics.tile([P, DC, IN_LEN], BF16)      # [d%, dc, i]
        for dc in range(DC):
            pa = psum.tile([P, IN_LEN], F32, tag="mm", name=f"pa_{dc}")
            for ac in range(AC):
                nc.tensor.matmul(
                    pa[:],
                    attn_w_bf[:, ac, dc * P : (dc + 1) * P],
                    ctxT_bf[:, ac, :],
                    start=(ac == 0),
                    stop=(ac == AC - 1),
                )
            nc.vector.tensor_scalar_add(ATb[:, dc, :], pa[:], attn_bias[:, dc : dc + 1])

        OTb = statics.tile([P, DC, OUT_LEN], F32)      # [d%, dc, o]
        for dc in range(DC):
            po = psum.tile([P, OUT_LEN], F32, tag="sm", name=f"po_{dc}")
            for ec in range(EC):
                nc.tensor.matmul(
                    po[:],
                    dec_w_sb[:, ec, dc * P : (dc + 1) * P],
                    combT_sb[:, EC + ec, :],
                    start=(ec == 0),
                    stop=(ec == EC - 1),
                )
            nc.vector.tensor_scalar_add(OTb[:, dc, :], po[:], dec_bias[:, dc : dc + 1])

        # out_w lands during the main loop (needed first by epilogue half 0)
        for cc in range(CC):
            for half in range(2):
                nc.sync.dma_start(
                    out_w_sb[:, cc, half * 256 : (half + 1) * 256],
                    out_w_d[cc * P : (cc + 1) * P, half * 256 : (half + 1) * 256],
                )

        # ---------------- main loop: tanh + q-reduction ----------------
        scores_sb = statics.tile([OUT_LEN, IN_LEN], F32)
        exp_sb = statics.tile([OUT_LEN, IN_LEN], F32)
        sums = statics.tile([OUT_LEN, 1], F32)
        recip = statics.tile([OUT_LEN, 1], F32)
        attn_sb = statics.tile([OUT_LEN, IN_LEN], F32)
        attnT_bf = statics.tile([P, IC, OUT_LEN], BF16)
        outT_sb = statics.tile([P, DC, OUT_LEN], F32)
        out_sb = statics.tile([OUT_LEN, DEC], F32)

        epi_args = (ident, scores_sb, exp_sb, sums, recip, attn_sb, attnT_bf,
                    ctx_bf, combT_sb, out_w_sb, out_bias, outT_sb, out_sb,
                    psum, attn_d, out_d)

        for og in range(NG):
            ps8 = psum.tile([G, IN_LEN], F32, tag="sc", name=f"ps8_{og}")
            for dc in range(DC):
                E = epool.tile([P, G, IN_LEN], BF16, tag="E", name=f"E_{og}_{dc}")
                for j in range(G):
                    o = og * G + j
                    nc.vector.tensor_scalar_add(
                        E[:, j, :], ATb[:, dc, :], OTb[:, dc, o : o + 1]
                    )
                Fc = fpool.tile([P, G, IN_LEN], BF16, tag="F", name=f"F_{og}_{dc}")
                nc.scalar.activation(Fc[:], E[:], AF.Tanh)
                for j in range(G):
                    nc.tensor.matmul(
                        ps8[:],
                        QZ[:, dc, j],
                        Fc[:, j],
                        start=(dc == 0 and j == 0),
                        stop=(dc == DC - 1 and j == G - 1),
                    )
            stage8 = spool.tile([G, IN_LEN], F32, tag="st", name=f"stage8_{og}")
            nc.vector.tensor_copy(stage8[:], ps8[:])
            nc.sync.dma_start(scores_sb[og * G : (og + 1) * G, :], stage8[:])

            if og == NG // 2 - 1:
                # rows 0..31 complete: overlap their softmax/mix/projection
                # with groups 4..7
                _epilogue_half(nc, 0, *epi_args)

        _epilogue_half(nc, 1, *epi_args)


_CACHE = {}


def build_nc():
    if "nc" in _CACHE:
        return _CACHE["nc"]
    nc = bacc.Bacc(
        "TRN2",
        target_bir_lowering=False,
        debug=False,
        num_devices=N_CORES,
    )
    with tile.TileContext(nc) as tc:
        _build_body(tc)
    nc.compile()
    _CACHE["nc"] = nc
    return nc


def kernel(**inputs):
    nc = build_nc()

    f = lambda k: np.ascontiguousarray(np.asarray(inputs[k], dtype=np.float32))
    output = f("output")
    context = f("context")
    shared = {
        "dec_w": f("dec_w"),
        "dec_b": f("dec_b").reshape(DEC, 1),
        "attn_w": f("attn_w"),
        "attn_b": f("attn_b").reshape(ATTN, 1),
        "query_w": f("query_w").reshape(DEC, 1),
        "out_w": f("out_w"),
        "out_b": f("out_b").reshape(DEC, 1),
    }
    in_maps = []
    for b in range(N_CORES):
        m = dict(shared)
        m["output"] = np.ascontiguousarray(output[b])
        m["context"] = np.ascontiguousarray(context[b])
        in_maps.append(m)

    res = bass_utils.run_bass_kernel_spmd(nc, in_maps, core_ids=list(range(N_CORES)))
    _CACHE["last_results"] = res
    out = np.stack([res.results[b]["out"] for b in range(N_CORES)])
    attn = np.stack([res.results[b]["attn"] for b in range(N_CORES)])
    return out, attn


# revision 11
# speedup vs baseline: 1.1395x; 1.1213x over previous
"""Bass/Tile Trainium2 kernel for additive (Bahdanau/'cat') attention.

Problem (per batch b):
  A[i,d]      = sum_a context[i,a] * attn_w[a,d] + attn_b[d]
  O[o,d]      = sum_e output[o,e]  * dec_w[e,d]  + dec_b[d]
  scores[o,i] = sum_d query_w[d] * tanh(A[i,d] + O[o,d])   (+query_b: softmax-invariant)
  attn        = softmax_i(scores)
  mix[o,a]    = sum_i attn[o,i] * context[i,a]
  out[o,d]    = tanh([mix | output] @ out_w + out_b)

Sharding: pure data-parallel over batch, B=8 -> one batch per NeuronCore,
weights broadcast, no collectives.

Per-core structure:
  * A^T [d,i] and O^T [d,o] kept with d on partitions so the broadcast add
    A^T + O^T[:,o] is a DVE tensor_scalar (per-partition scalar), in bf16.
  * tanh batched 8 o's per ACT instruction (free dim 4096); d-chunk-outer
    so the PE gets matmul work after every ACT chunk (keeps HAM warm).
  * q-reduction over d on the PE with zero-padded stationary operand:
    lhsT QZ[:,dc,j] is [128,8] holding query_w in column j -> all 32
    matmuls of a group accumulate into ONE [8,512] PSUM bank; one cheap
    8-row DVE copy + SBUF->SBUF DMA scatters rows into scores.
  * softmax/mix/out epilogue computed in row-halves (0:32 during groups
    4..7, 32:64 at the end) to shorten the serial tail.
"""

import numpy as np

import concourse.bass as bass
import concourse.tile as tile
import concourse.bass_utils as bass_utils
from concourse import bacc, mybir
from concourse.masks import make_identity

B, OUT_LEN, IN_LEN, DEC, ATTN = 8, 64, 512, 512, 512
P = 128
F32 = mybir.dt.float32
BF16 = mybir.dt.bfloat16
AF = mybir.ActivationFunctionType

G = 8                     # o's per tanh group
NG = OUT_LEN // G         # 8 groups
DC = DEC // P             # 4 d-chunks
AC = ATTN // P            # 4 a-chunks
IC = IN_LEN // P          # 4 i-chunks
EC = DEC // P             # 4 e-chunks (decoder feature)
CC = (ATTN + DEC) // P    # 8 combined chunks
H = OUT_LEN // 2          # row half

N_CORES = 8


def _epilogue_softmax_mix(nc, h, ident, scores_sb, exp_sb, sums, recip, attn_sb,
                          attnT_bf, ctx_bf, combT_sb, psum, attn_d):
    """softmax + attn^T + mix for rows h*32..h*32+31."""
    r0 = h * H
    sl = slice(r0, r0 + H)
    nc.scalar.activation(exp_sb[sl, :], scores_sb[sl, :], AF.Exp, accum_out=sums[sl, :])
    nc.vector.reciprocal(recip[sl, :], sums[sl, :])
    nc.vector.tensor_scalar_mul(attn_sb[sl, :], exp_sb[sl, :], recip[sl, :])
    nc.sync.dma_start(attn_d[sl, :], attn_sb[sl, :])

    # attn^T (converted to bf16 in the PSUM evac) for this half
    for ic in range(IC):
        pt = psum.tile([P, H], F32, tag="tp", name=f"pt_at_{h}_{ic}")
        nc.tensor.transpose(
            pt[:], attn_sb[sl, ic * P : (ic + 1) * P], ident[sl, r0 : r0 + H]
        )
        nc.vector.tensor_copy(attnT_bf[:, ic, sl], pt[:])

    # mix^T -> combined chunks 0..3
    for ac in range(AC):
        pm = psum.tile([P, H], F32, tag="sm", name=f"pm_{h}_{ac}")
        for ic in range(IC):
            nc.tensor.matmul(
                pm[:],
                ctx_bf[:, ic, ac * P : (ac + 1) * P],
                attnT_bf[:, ic, sl],
                start=(ic == 0),
                stop=(ic == IC - 1),
            )
        nc.vector.tensor_copy(combT_sb[:, ac, sl], pm[:])


def _epilogue_project(nc, h, ident, combT_sb, out_w_sb, out_bias, outT_sb,
                      out_sb, psum, out_d):
    """output projection + store for rows h*32..h*32+31."""
    r0 = h * H
    sl = slice(r0, r0 + H)
    for dc in range(DC):
        pf = psum.tile([P, H], F32, tag="sm", name=f"pf_{h}_{dc}")
        for cc in range(CC):
            nc.tensor.matmul(
                pf[:],
                out_w_sb[:, cc, dc * P : (dc + 1) * P],
                combT_sb[:, cc, sl],
                start=(cc == 0),
                stop=(cc == CC - 1),
            )
        nc.scalar.activation(
            outT_sb[:, dc, sl], pf[:], AF.Tanh, bias=out_bias[:, dc : dc + 1]
        )

    for dc in range(DC):
        pt = psum.tile([H, P], F32, tag="tp", name=f"pt_o_{h}_{dc}")
        nc.tensor.transpose(pt[:], outT_sb[:, dc, sl], ident[:])
        nc.vector.tensor_copy(out_sb[sl, dc * P : (dc + 1) * P], pt[:])
    nc.sync.dma_start(out_d[sl, :], out_sb[sl, :])


def _build_body(tc):
    nc = tc.nc

    # ---- DRAM I/O (per-core shard shapes) ----
    output_d = nc.dram_tensor("output", [OUT_LEN, DEC], F32, kind="ExternalInput").ap()
    context_d = nc.dram_tensor("context", [IN_LEN, ATTN], F32, kind="ExternalInput").ap()
    dec_w_d = nc.dram_tensor("dec_w", [DEC, DEC], F32, kind="ExternalInput").ap()
    dec_b_d = nc.dram_tensor("dec_b", [DEC, 1], F32, kind="ExternalInput").ap()
    attn_w_d = nc.dram_tensor("attn_w", [ATTN, DEC], F32, kind="ExternalInput").ap()
    attn_b_d = nc.dram_tensor("attn_b", [ATTN, 1], F32, kind="ExternalInput").ap()
    query_w_d = nc.dram_tensor("query_w", [DEC, 1], F32, kind="ExternalInput").ap()
    out_w_d = nc.dram_tensor("out_w", [ATTN + DEC, DEC], F32, kind="ExternalInput").ap()
    out_b_d = nc.dram_tensor("out_b", [DEC, 1], F32, kind="ExternalInput").ap()
    out_d = nc.dram_tensor("out", [OUT_LEN, DEC], F32, kind="ExternalOutput").ap()
    attn_d = nc.dram_tensor("attn", [OUT_LEN, IN_LEN], F32, kind="ExternalOutput").ap()

    from contextlib import ExitStack

    with ExitStack() as ctx:
        const = ctx.enter_context(tc.tile_pool(name="const", bufs=1))
        statics = ctx.enter_context(tc.tile_pool(name="statics", bufs=1))
        epool = ctx.enter_context(tc.tile_pool(name="epool", bufs=4))
        fpool = ctx.enter_context(tc.tile_pool(name="fpool", bufs=3))
        spool = ctx.enter_context(tc.tile_pool(name="spool", bufs=2))
        psum = ctx.enter_context(tc.tile_pool(name="psum", bufs=2, space="PSUM"))

        # ---------------- constants / small inputs ----------------
        ident = const.tile([P, P], F32)
        make_identity(nc, ident)
        ident_bf = const.tile([P, P], BF16)
        nc.vector.tensor_copy(ident_bf[:], ident[:])

        # HAM warmup: ~4.5us of real matmul activity on dummy data flips the
        # PE clock gate to 8/8 (2.4 GHz) before the real matmuls arrive.
        # (PE-transpose-mode does not count as HAM activity.)
        wu = psum.tile([P, P], F32, tag="mm")
        for _ in range(44):
            nc.tensor.matmul(wu[:], ident_bf[:], ident_bf[:], start=True, stop=True)

        attn_bias = const.tile([P, DC], F32)
        dec_bias = const.tile([P, DC], F32)
        out_bias = const.tile([P, DC], F32)
        qw_f = const.tile([P, DC], F32)
        qw_bf = const.tile([P, DC], BF16)
        for tile_, dram_ in ((attn_bias, attn_b_d), (dec_bias, dec_b_d),
                             (out_bias, out_b_d), (qw_f, query_w_d)):
            nc.scalar.dma_start(
                tile_[:], dram_.rearrange("(dc p) one -> p dc one", p=P)
            )
        nc.vector.tensor_copy(qw_bf[:], qw_f[:])

        # zero-padded stationary operands: QZ[:, dc, j] is [128, G] with
        # query_w[dc] in column j, zeros elsewhere -> matmul j deposits
        # scores for o_j into PSUM row j, rows != j accumulate zeros.
        QZ = const.tile([P, DC, G, G], BF16)
        nc.vector.memset(QZ[:], 0.0)
        for dc in range(DC):
            for j in range(G):
                nc.vector.tensor_copy(QZ[:, dc, j, j : j + 1], qw_bf[:, dc : dc + 1])

        # ---------------- big input DMAs (split for queue parallelism) ----
        ctx_sb = statics.tile([P, IC, ATTN], F32)      # [i%, ic, a]
        attn_w_sb = statics.tile([P, AC, DEC], F32)    # [a%, ac, d]
        dec_w_sb = statics.tile([P, EC, DEC], F32)     # [e%, ec, d]
        output_sb = statics.tile([OUT_LEN, DEC], F32)  # [o, e]
        out_w_sb = statics.tile([P, CC, DEC], F32)     # [c%, cc, d]
        for ic in range(IC):
            nc.sync.dma_start(ctx_sb[:, ic, :], context_d[ic * P : (ic + 1) * P, :])
        nc.scalar.dma_start(output_sb[:], output_d[:])
        for ac in range(AC):
            nc.scalar.dma_start(attn_w_sb[:, ac, :], attn_w_d[ac * P : (ac + 1) * P, :])
        for ec in range(EC):
            nc.sync.dma_start(dec_w_sb[:, ec, :], dec_w_d[ec * P : (ec + 1) * P, :])

        # bf16 copies for the attention-score pipeline
        ctx_bf = statics.tile([P, IC, ATTN], BF16)
        attn_w_bf = statics.tile([P, AC, DEC], BF16)
        for ic in range(IC):
            nc.vector.tensor_copy(ctx_bf[:, ic, :], ctx_sb[:, ic, :])
        for ac in range(AC):
            nc.vector.tensor_copy(attn_w_bf[:, ac, :], attn_w_sb[:, ac, :])

        # ---------------- transposes: context^T (bf16), output^T ----------
        ctxT_bf = statics.tile([P, AC, IN_LEN], BF16)  # [a%, ac, i]
        for ic in range(IC):
            for ac in range(AC):
                pt = psum.tile([P, P], BF16, tag="tp", name=f"pt_c_{ic}_{ac}")
                nc.tensor.transpose(pt[:], ctx_bf[:, ic, ac * P : (ac + 1) * P], ident_bf[:])
                nc.vector.tensor_copy(ctxT_bf[:, ac, ic * P : (ic + 1) * P], pt[:])

        # combined^T [c%, cc, o]: chunks 0..3 = mix^T (later), 4..7 = output^T
        combT_sb = statics.tile([P, CC, OUT_LEN], F32)
        for ec in range(EC):
            pt = psum.tile([P, OUT_LEN], F32, tag="tp", name=f"pt_ot_{ec}")
            nc.tensor.transpose(
                pt[:], output_sb[0:OUT_LEN, ec * P : (ec + 1) * P],
                ident[0:OUT_LEN, 0:OUT_LEN],
            )
            nc.vector.tensor_copy(combT_sb[:, EC + ec, :], pt[:])

        # ---------------- A^T (bf16 matmul) and O^T ----------------
        ATb = statics.tile([P, DC, IN_LEN], BF16)      # [d%, dc, i]
        for dc in range(DC):
            pa = psum.tile([P, IN_LEN], F32, tag="mm", name=f"pa_{dc}")
            for ac in range(AC):
                nc.tensor.matmul(
                    pa[:],
                    attn_w_bf[:, ac, dc * P : (dc + 1) * P],
                    ctxT_bf[:, ac, :],
                    start=(ac == 0),
                    stop=(ac == AC - 1),
                )
            nc.vector.tensor_scalar_add(ATb[:, dc, :], pa[:], attn_bias[:, dc : dc + 1])

        OTb = statics.tile([P, DC, OUT_LEN], F32)      # [d%, dc, o]
        for dc in range(DC):
            po = psum.tile([P, OUT_LEN], F32, tag="sm", name=f"po_{dc}")
            for ec in range(EC):
                nc.tensor.matmul(
                    po[:],
                    dec_w_sb[:, ec, dc * P : (dc + 1) * P],
                    combT_sb[:, EC + ec, :],
                    start=(ec == 0),
                    stop=(ec == EC - 1),
                )
            nc.vector.tensor_scalar_add(OTb[:, dc, :], po[:], dec_bias[:, dc : dc + 1])

        # out_w lands during the main loop (needed first by epilogue half 0)
        for cc in range(CC):
            nc.sync.dma_start(out_w_sb[:, cc, :], out_w_d[cc * P : (cc + 1) * P, :])

        # ---------------- main loop: tanh + q-reduction ----------------
        scores_sb = statics.tile([OUT_LEN, IN_LEN], F32)
        exp_sb = statics.tile([OUT_LEN, IN_LEN], F32)
        sums = statics.tile([OUT_LEN, 1], F32)
        recip = statics.tile([OUT_LEN, 1], F32)
        attn_sb = statics.tile([OUT_LEN, IN_LEN], F32)
        attnT_bf = statics.tile([P, IC, OUT_LEN], BF16)
        outT_sb = statics.tile([P, DC, OUT_LEN], F32)
        out_sb = statics.tile([OUT_LEN, DEC], F32)

        sm_args = (ident, scores_sb, exp_sb, sums, recip, attn_sb, attnT_bf,
                   ctx_bf, combT_sb, psum, attn_d)
        pr_args = (ident, combT_sb, out_w_sb, out_bias, outT_sb, out_sb,
                   psum, out_d)

        for og in range(NG):
            ps8 = psum.tile([G, IN_LEN], F32, tag="sc", name=f"ps8_{og}")
            for dc in range(DC):
                E = epool.tile([P, G, IN_LEN], BF16, tag="E", name=f"E_{og}_{dc}")
                for j in range(G):
                    o = og * G + j
                    nc.vector.tensor_scalar_add(
                        E[:, j, :], ATb[:, dc, :], OTb[:, dc, o : o + 1]
                    )
                Fc = fpool.tile([P, G, IN_LEN], BF16, tag="F", name=f"F_{og}_{dc}")
                nc.scalar.activation(Fc[:], E[:], AF.Tanh)
                for j in range(G):
                    nc.tensor.matmul(
                        ps8[:],
                        QZ[:, dc, j],
                        Fc[:, j],
                        start=(dc == 0 and j == 0),
                        stop=(dc == DC - 1 and j == G - 1),
                    )
            stage8 = spool.tile([G, IN_LEN], F32, tag="st", name=f"stage8_{og}")
            nc.vector.tensor_copy(stage8[:], ps8[:])
            nc.sync.dma_start(scores_sb[og * G : (og + 1) * G, :], stage8[:])

            if og == NG // 2:
                # rows 0..31 complete since og 3: their softmax + mix runs
                # under og 5..7 (placed here so the ACT stream never blocks)
                _epilogue_softmax_mix(nc, 0, *sm_args)
            elif og == NG // 2 + 1:
                _epilogue_project(nc, 0, *pr_args)

        _epilogue_softmax_mix(nc, 1, *sm_args)
        _epilogue_project(nc, 1, *pr_args)


_CACHE = {}


def build_nc():
    if "nc" in _CACHE:
        return _CACHE["nc"]
    nc = bacc.Bacc(
        "TRN2",
        target_bir_lowering=False,
        debug=False,
        num_devices=N_CORES,
    )
    with tile.TileContext(nc) as tc:
        _build_body(tc)
    nc.compile()
    _CACHE["nc"] = nc
    return nc


def kernel(**inputs):
    nc = build_nc()

    f = lambda k: np.ascontiguousarray(np.asarray(inputs[k], dtype=np.float32))
    output = f("output")
    context = f("context")
    shared = {
        "dec_w": f("dec_w"),
        "dec_b": f("dec_b").reshape(DEC, 1),
        "attn_w": f("attn_w"),
        "attn_b": f("attn_b").reshape(ATTN, 1),
        "query_w": f("query_w").reshape(DEC, 1),
        "out_w": f("out_w"),
        "out_b": f("out_b").reshape(DEC, 1),
    }
    in_maps = []
    for b in range(N_CORES):
        m = dict(shared)
        m["output"] = np.ascontiguousarray(output[b])
        m["context"] = np.ascontiguousarray(context[b])
        in_maps.append(m)

    res = bass_utils.run_bass_kernel_spmd(nc, in_maps, core_ids=list(range(N_CORES)))
    _CACHE["last_results"] = res
    out = np.stack([res.results[b]["out"] for b in range(N_CORES)])
    attn = np.stack([res.results[b]["attn"] for b in range(N_CORES)])
    return out, attn


# revision 12
# speedup vs baseline: 1.1855x; 1.0404x over previous
"""Bass/Tile Trainium2 kernel for additive (Bahdanau/'cat') attention.

Problem (per batch b):
  A[i,d]      = sum_a context[i,a] * attn_w[a,d] + attn_b[d]
  O[o,d]      = sum_e output[o,e]  * dec_w[e,d]  + dec_b[d]
  scores[o,i] = sum_d query_w[d] * tanh(A[i,d] + O[o,d])   (+query_b: softmax-invariant)
  attn        = softmax_i(scores)
  mix[o,a]    = sum_i attn[o,i] * context[i,a]
  out[o,d]    = tanh([mix | output] @ out_w + out_b)

Sharding: pure data-parallel over batch, B=8 -> one batch per NeuronCore,
weights broadcast, no collectives.

Per-core structure:
  * A^T [d,i] and O^T [d,o] kept with d on partitions so the broadcast add
    A^T + O^T[:,o] is a DVE tensor_scalar (per-partition scalar), in bf16.
  * tanh batched 8 o's per ACT instruction (free dim 4096); d-chunk-outer
    so the PE gets matmul work after every ACT chunk (keeps HAM warm).
  * q-reduction over d on the PE with zero-padded stationary operand:
    lhsT QZ[:,dc,j] is [128,8] holding query_w in column j -> all 32
    matmuls of a group accumulate into ONE [8,512] PSUM bank; one cheap
    8-row DVE copy + SBUF->SBUF DMA scatters rows into scores.
  * softmax/mix/out epilogue computed in row-halves (0:32 during groups
    4..7, 32:64 at the end) to shorten the serial tail.
"""

import numpy as np

import concourse.bass as bass
import concourse.tile as tile
import concourse.bass_utils as bass_utils
from concourse import bacc, mybir
from concourse.masks import make_identity

B, OUT_LEN, IN_LEN, DEC, ATTN = 8, 64, 512, 512, 512
P = 128
F32 = mybir.dt.float32
BF16 = mybir.dt.bfloat16
AF = mybir.ActivationFunctionType

G = 8                     # o's per tanh group
NG = OUT_LEN // G         # 8 groups
DC = DEC // P             # 4 d-chunks
AC = ATTN // P            # 4 a-chunks
IC = IN_LEN // P          # 4 i-chunks
EC = DEC // P             # 4 e-chunks (decoder feature)
CC = (ATTN + DEC) // P    # 8 combined chunks
H = OUT_LEN // 2          # row half

N_CORES = 8


def _epilogue_softmax_mix(nc, h, ident_bf, scores_sb, exp_sb, sums, recip,
                          attn_sb, attn_bf, attnT_bf, ctx_bf, combT_bf, psum,
                          attn_d):
    """softmax + attn^T + mix for rows h*32..h*32+31 (all-bf16 matmuls)."""
    r0 = h * H
    sl = slice(r0, r0 + H)
    nc.scalar.activation(exp_sb[sl, :], scores_sb[sl, :], AF.Exp, accum_out=sums[sl, :])
    nc.vector.reciprocal(recip[sl, :], sums[sl, :])
    nc.vector.tensor_scalar_mul(attn_sb[sl, :], exp_sb[sl, :], recip[sl, :])
    nc.sync.dma_start(attn_d[sl, :], attn_sb[sl, :])
    nc.vector.tensor_copy(attn_bf[sl, :], attn_sb[sl, :])

    for ic in range(IC):
        pt = psum.tile([P, H], BF16, tag="tp", name=f"pt_at_{h}_{ic}")
        nc.tensor.transpose(
            pt[:], attn_bf[sl, ic * P : (ic + 1) * P], ident_bf[sl, r0 : r0 + H]
        )
        nc.vector.tensor_copy(attnT_bf[:, ic, sl], pt[:])

    # mix^T -> combined chunks 0..3
    for ac in range(AC):
        pm = psum.tile([P, H], F32, tag="sm", name=f"pm_{h}_{ac}")
        for ic in range(IC):
            nc.tensor.matmul(
                pm[:],
                ctx_bf[:, ic, ac * P : (ac + 1) * P],
                attnT_bf[:, ic, sl],
                start=(ic == 0),
                stop=(ic == IC - 1),
            )
        nc.vector.tensor_copy(combT_bf[:, ac, sl], pm[:])


def _final_project(nc, combT_bf, out_w_bf, ones_bf, outb_row_bf, out_sb, psum,
                   out_d):
    """out = tanh(combined @ out_w + out_b) for all 64 rows at once:
    M=64 x N=512 matmuls, bias applied as a rank-1 (K=1) accumulation."""
    po = psum.tile([OUT_LEN, DEC], F32, tag="mm", name="po_final")
    for cc in range(CC):
        nc.tensor.matmul(
            po[:], combT_bf[:, cc, :], out_w_bf[:, cc, :],
            start=(cc == 0), stop=False,
        )
    nc.tensor.matmul(po[:], ones_bf[:], outb_row_bf[:], start=False, stop=True)
    nc.scalar.activation(out_sb[:], po[:], AF.Tanh)
    nc.sync.dma_start(out_d[:], out_sb[:])


def _build_body(tc):
    nc = tc.nc

    # ---- DRAM I/O (per-core shard shapes) ----
    output_d = nc.dram_tensor("output", [OUT_LEN, DEC], F32, kind="ExternalInput").ap()
    context_d = nc.dram_tensor("context", [IN_LEN, ATTN], F32, kind="ExternalInput").ap()
    dec_w_d = nc.dram_tensor("dec_w", [DEC, DEC], F32, kind="ExternalInput").ap()
    dec_b_d = nc.dram_tensor("dec_b", [DEC, 1], F32, kind="ExternalInput").ap()
    attn_w_d = nc.dram_tensor("attn_w", [ATTN, DEC], F32, kind="ExternalInput").ap()
    attn_b_d = nc.dram_tensor("attn_b", [ATTN, 1], F32, kind="ExternalInput").ap()
    query_w_d = nc.dram_tensor("query_w", [DEC, 1], F32, kind="ExternalInput").ap()
    out_w_d = nc.dram_tensor("out_w", [ATTN + DEC, DEC], F32, kind="ExternalInput").ap()
    out_b_d = nc.dram_tensor("out_b", [DEC, 1], F32, kind="ExternalInput").ap()
    out_d = nc.dram_tensor("out", [OUT_LEN, DEC], F32, kind="ExternalOutput").ap()
    attn_d = nc.dram_tensor("attn", [OUT_LEN, IN_LEN], F32, kind="ExternalOutput").ap()

    from contextlib import ExitStack

    with ExitStack() as ctx:
        const = ctx.enter_context(tc.tile_pool(name="const", bufs=1))
        statics = ctx.enter_context(tc.tile_pool(name="statics", bufs=1))
        epool = ctx.enter_context(tc.tile_pool(name="epool", bufs=4))
        fpool = ctx.enter_context(tc.tile_pool(name="fpool", bufs=3))
        spool = ctx.enter_context(tc.tile_pool(name="spool", bufs=2))
        psum = ctx.enter_context(tc.tile_pool(name="psum", bufs=2, space="PSUM"))

        # ---------------- constants / small inputs ----------------
        ident = const.tile([P, P], F32)
        make_identity(nc, ident)
        ident_bf = const.tile([P, P], BF16)
        nc.vector.tensor_copy(ident_bf[:], ident[:])

        # HAM warmup: ~4.5us of real matmul activity on dummy data flips the
        # PE clock gate to 8/8 (2.4 GHz) before the real matmuls arrive.
        # (PE-transpose-mode does not count as HAM activity.)
        wu = psum.tile([P, P], F32, tag="mm")
        for _ in range(44):
            nc.tensor.matmul(wu[:], ident_bf[:], ident_bf[:], start=True, stop=True)

        attn_bias = const.tile([P, DC], F32)
        dec_bias = const.tile([P, DC], F32)
        qw_f = const.tile([P, DC], F32)
        qw_bf = const.tile([P, DC], BF16)
        for tile_, dram_ in ((attn_bias, attn_b_d), (dec_bias, dec_b_d),
                             (qw_f, query_w_d)):
            nc.scalar.dma_start(
                tile_[:], dram_.rearrange("(dc p) one -> p dc one", p=P)
            )
        nc.vector.tensor_copy(qw_bf[:], qw_f[:])

        ones_bf = const.tile([1, OUT_LEN], BF16)
        nc.vector.memset(ones_bf[:], 1.0)
        outb_row_f = const.tile([1, DEC], F32)
        nc.scalar.dma_start(outb_row_f[:], out_b_d.rearrange("d one -> one d"))
        outb_row_bf = const.tile([1, DEC], BF16)
        nc.vector.tensor_copy(outb_row_bf[:], outb_row_f[:])

        # zero-padded stationary operands: QZ[:, dc, j] is [128, G] with
        # query_w[dc] in column j, zeros elsewhere -> matmul j deposits
        # scores for o_j into PSUM row j, rows != j accumulate zeros.
        QZ = const.tile([P, DC, G, G], BF16)
        nc.vector.memset(QZ[:], 0.0)
        for dc in range(DC):
            for j in range(G):
                nc.vector.tensor_copy(QZ[:, dc, j, j : j + 1], qw_bf[:, dc : dc + 1])

        # ---------------- big input DMAs (split for queue parallelism) ----
        ctx_sb = statics.tile([P, IC, ATTN], F32)      # [i%, ic, a]
        attn_w_sb = statics.tile([P, AC, DEC], F32)    # [a%, ac, d]
        dec_w_sb = statics.tile([P, EC, DEC], F32)     # [e%, ec, d]
        output_sb = statics.tile([OUT_LEN, DEC], F32)  # [o, e]
        out_w_sb = statics.tile([P, CC, DEC], F32)     # [c%, cc, d]
        for ic in range(IC):
            nc.sync.dma_start(ctx_sb[:, ic, :], context_d[ic * P : (ic + 1) * P, :])
        nc.scalar.dma_start(output_sb[:], output_d[:])
        for ac in range(AC):
            nc.scalar.dma_start(attn_w_sb[:, ac, :], attn_w_d[ac * P : (ac + 1) * P, :])
        for ec in range(EC):
            nc.sync.dma_start(dec_w_sb[:, ec, :], dec_w_d[ec * P : (ec + 1) * P, :])

        # paced HAM keep-alive: dummy matmuls that depend on arriving DMA
        # data keep the PE busy-window alive until the real matmuls start
        for ic in range(IC):
            wuc = psum.tile([P, ATTN], F32, tag="mm", name=f"wuc_{ic}")
            nc.tensor.matmul(wuc[:], ident[:], ctx_sb[:, ic, :], start=True, stop=True)
        for ec in range(EC):
            wud = psum.tile([P, DEC], F32, tag="mm", name=f"wud_{ec}")
            nc.tensor.matmul(wud[:], ident[:], dec_w_sb[:, ec, :], start=True, stop=True)

        # bf16 copies for the attention-score pipeline
        ctx_bf = statics.tile([P, IC, ATTN], BF16)
        attn_w_bf = statics.tile([P, AC, DEC], BF16)
        dec_w_bf = statics.tile([P, EC, DEC], BF16)
        out_w_bf = statics.tile([P, CC, DEC], BF16)
        output_bf = statics.tile([OUT_LEN, DEC], BF16)
        for ic in range(IC):
            nc.vector.tensor_copy(ctx_bf[:, ic, :], ctx_sb[:, ic, :])
        for ac in range(AC):
            nc.vector.tensor_copy(attn_w_bf[:, ac, :], attn_w_sb[:, ac, :])
        for ec in range(EC):
            nc.vector.tensor_copy(dec_w_bf[:, ec, :], dec_w_sb[:, ec, :])
        nc.vector.tensor_copy(output_bf[:], output_sb[:])

        # ---------------- transposes: context^T (bf16), output^T ----------
        ctxT_bf = statics.tile([P, AC, IN_LEN], BF16)  # [a%, ac, i]
        for ic in range(IC):
            for ac in range(AC):
                pt = psum.tile([P, P], BF16, tag="tp", name=f"pt_c_{ic}_{ac}")
                nc.tensor.transpose(pt[:], ctx_bf[:, ic, ac * P : (ac + 1) * P], ident_bf[:])
                nc.vector.tensor_copy(ctxT_bf[:, ac, ic * P : (ic + 1) * P], pt[:])

        # combined^T [c%, cc, o]: chunks 0..3 = mix^T (later), 4..7 = output^T
        combT_bf = statics.tile([P, CC, OUT_LEN], BF16)
        for ec in range(EC):
            pt = psum.tile([P, OUT_LEN], BF16, tag="tp", name=f"pt_ot_{ec}")
            nc.tensor.transpose(
                pt[:], output_bf[0:OUT_LEN, ec * P : (ec + 1) * P],
                ident_bf[0:OUT_LEN, 0:OUT_LEN],
            )
            nc.vector.tensor_copy(combT_bf[:, EC + ec, :], pt[:])

        # ---------------- A^T (bf16 matmul) and O^T ----------------
        ATb = statics.tile([P, DC, IN_LEN], BF16)      # [d%, dc, i]
        for dc in range(DC):
            pa = psum.tile([P, IN_LEN], F32, tag="mm", name=f"pa_{dc}")
            for ac in range(AC):
                nc.tensor.matmul(
                    pa[:],
                    attn_w_bf[:, ac, dc * P : (dc + 1) * P],
                    ctxT_bf[:, ac, :],
                    start=(ac == 0),
                    stop=(ac == AC - 1),
                )
            nc.vector.tensor_scalar_add(ATb[:, dc, :], pa[:], attn_bias[:, dc : dc + 1])

        OTb = statics.tile([P, DC, OUT_LEN], F32)      # [d%, dc, o]
        for dc in range(DC):
            po = psum.tile([P, OUT_LEN], F32, tag="sm", name=f"po_{dc}")
            for ec in range(EC):
                nc.tensor.matmul(
                    po[:],
                    dec_w_bf[:, ec, dc * P : (dc + 1) * P],
                    combT_bf[:, EC + ec, :],
                    start=(ec == 0),
                    stop=(ec == EC - 1),
                )
            nc.vector.tensor_scalar_add(OTb[:, dc, :], po[:], dec_bias[:, dc : dc + 1])

        # out_w lands during the main loop (needed first by epilogue half 0)
        for cc in range(CC):
            nc.sync.dma_start(out_w_sb[:, cc, :], out_w_d[cc * P : (cc + 1) * P, :])
        for cc in range(CC):
            nc.vector.tensor_copy(out_w_bf[:, cc, :], out_w_sb[:, cc, :])

        # ---------------- main loop: tanh + q-reduction ----------------
        scores_sb = statics.tile([OUT_LEN, IN_LEN], F32)
        exp_sb = statics.tile([OUT_LEN, IN_LEN], F32)
        sums = statics.tile([OUT_LEN, 1], F32)
        recip = statics.tile([OUT_LEN, 1], F32)
        attn_sb = statics.tile([OUT_LEN, IN_LEN], F32)
        attn_bf = statics.tile([OUT_LEN, IN_LEN], BF16)
        attnT_bf = statics.tile([P, IC, OUT_LEN], BF16)
        out_sb = statics.tile([OUT_LEN, DEC], F32)

        sm_args = (ident_bf, scores_sb, exp_sb, sums, recip, attn_sb, attn_bf,
                   attnT_bf, ctx_bf, combT_bf, psum, attn_d)

        for og in range(NG):
            ps8 = psum.tile([G, IN_LEN], F32, tag="sc", name=f"ps8_{og}")
            for dc in range(DC):
                E = epool.tile([P, G, IN_LEN], BF16, tag="E", name=f"E_{og}_{dc}")
                for j in range(G):
                    o = og * G + j
                    nc.vector.tensor_scalar_add(
                        E[:, j, :], ATb[:, dc, :], OTb[:, dc, o : o + 1]
                    )
                Fc = fpool.tile([P, G, IN_LEN], BF16, tag="F", name=f"F_{og}_{dc}")
                nc.scalar.activation(Fc[:], E[:], AF.Tanh)
                for j in range(G):
                    nc.tensor.matmul(
                        ps8[:],
                        QZ[:, dc, j],
                        Fc[:, j],
                        start=(dc == 0 and j == 0),
                        stop=(dc == DC - 1 and j == G - 1),
                    )
            stage8 = spool.tile([G, IN_LEN], F32, tag="st", name=f"stage8_{og}")
            nc.vector.tensor_copy(stage8[:], ps8[:])
            nc.sync.dma_start(scores_sb[og * G : (og + 1) * G, :], stage8[:])

            if og == NG // 2:
                # rows 0..31 complete since og 3: their softmax + mix runs
                # under og 5..7 (placed here so the ACT stream never blocks)
                _epilogue_softmax_mix(nc, 0, *sm_args)

        _epilogue_softmax_mix(nc, 1, *sm_args)
        _final_project(nc, combT_bf, out_w_bf, ones_bf, outb_row_bf, out_sb,
                       psum, out_d)


_CACHE = {}


def build_nc():
    if "nc" in _CACHE:
        return _CACHE["nc"]
    nc = bacc.Bacc(
        "TRN2",
        target_bir_lowering=False,
        debug=False,
        num_devices=N_CORES,
    )
    with tile.TileContext(nc) as tc:
        _build_body(tc)
    nc.compile()
    _CACHE["nc"] = nc
    return nc


def kernel(**inputs):
    nc = build_nc()

    f = lambda k: np.ascontiguousarray(np.asarray(inputs[k], dtype=np.float32))
    output = f("output")
    context = f("context")
    shared = {
        "dec_w": f("dec_w"),
        "dec_b": f("dec_b").reshape(DEC, 1),
        "attn_w": f("attn_w"),
        "attn_b": f("attn_b").reshape(ATTN, 1),
        "query_w": f("query_w").reshape(DEC, 1),
        "out_w": f("out_w"),
        "out_b": f("out_b").reshape(DEC, 1),
    }
    in_maps = []
    for b in range(N_CORES):
        m = dict(shared)
        m["output"] = np.ascontiguousarray(output[b])
        m["context"] = np.ascontiguousarray(context[b])
        in_maps.append(m)

    res = bass_utils.run_bass_kernel_spmd(nc, in_maps, core_ids=list(range(N_CORES)))
    _CACHE["last_results"] = res
    out = np.stack([res.results[b]["out"] for b in range(N_CORES)])
    attn = np.stack([res.results[b]["attn"] for b in range(N_CORES)])
    return out, attn


# revision 13
# speedup vs baseline: 1.2122x; 1.0225x over previous
"""Bass/Tile Trainium2 kernel for additive (Bahdanau/'cat') attention.

Problem (per batch b):
  A[i,d]      = sum_a context[i,a] * attn_w[a,d] + attn_b[d]
  O[o,d]      = sum_e output[o,e]  * dec_w[e,d]  + dec_b[d]
  scores[o,i] = sum_d query_w[d] * tanh(A[i,d] + O[o,d])   (+query_b: softmax-invariant)
  attn        = softmax_i(scores)
  mix[o,a]    = sum_i attn[o,i] * context[i,a]
  out[o,d]    = tanh([mix | output] @ out_w + out_b)

Sharding: pure data-parallel over batch, B=8 -> one batch per NeuronCore,
weights broadcast, no collectives.

Per-core structure:
  * A^T [d,i] and O^T [d,o] kept with d on partitions so the broadcast add
    A^T + O^T[:,o] is a DVE tensor_scalar (per-partition scalar), in bf16.
  * tanh batched 8 o's per ACT instruction (free dim 4096); d-chunk-outer
    so the PE gets matmul work after every ACT chunk (keeps HAM warm).
  * q-reduction over d on the PE with zero-padded stationary operand:
    lhsT QZ[:,dc,j] is [128,8] holding query_w in column j -> all 32
    matmuls of a group accumulate into ONE [8,512] PSUM bank; one cheap
    8-row DVE copy + SBUF->SBUF DMA scatters rows into scores.
  * softmax/mix/out epilogue computed in row-halves (0:32 during groups
    4..7, 32:64 at the end) to shorten the serial tail.
"""

import numpy as np

import concourse.bass as bass
import concourse.tile as tile
import concourse.bass_utils as bass_utils
from concourse import bacc, mybir
from concourse.masks import make_identity

B, OUT_LEN, IN_LEN, DEC, ATTN = 8, 64, 512, 512, 512
P = 128
F32 = mybir.dt.float32
BF16 = mybir.dt.bfloat16
AF = mybir.ActivationFunctionType

G = 8                     # o's per tanh group
NG = OUT_LEN // G         # 8 groups
DC = DEC // P             # 4 d-chunks
AC = ATTN // P            # 4 a-chunks
IC = IN_LEN // P          # 4 i-chunks
EC = DEC // P             # 4 e-chunks (decoder feature)
CC = (ATTN + DEC) // P    # 8 combined chunks
H = OUT_LEN // 2          # row half

N_CORES = 8


def _epilogue_softmax_mix(nc, h, ident_bf, scores_sb, exp_sb, sums, recip,
                          attn_sb, attn_bf, attnT_bf, ctx_bf, combT_bf, psum,
                          attn_d):
    """softmax + attn^T + mix for rows h*32..h*32+31 (all-bf16 matmuls)."""
    r0 = h * H
    sl = slice(r0, r0 + H)
    nc.scalar.activation(exp_sb[sl, :], scores_sb[sl, :], AF.Exp, accum_out=sums[sl, :])
    nc.vector.reciprocal(recip[sl, :], sums[sl, :])
    nc.vector.tensor_scalar_mul(attn_sb[sl, :], exp_sb[sl, :], recip[sl, :])
    nc.sync.dma_start(attn_d[sl, :], attn_sb[sl, :])
    nc.vector.tensor_copy(attn_bf[sl, :], attn_sb[sl, :])

    for ic in range(IC):
        pt = psum.tile([P, H], BF16, tag="tp", name=f"pt_at_{h}_{ic}")
        nc.tensor.transpose(
            pt[:], attn_bf[sl, ic * P : (ic + 1) * P], ident_bf[sl, r0 : r0 + H]
        )
        nc.vector.tensor_copy(attnT_bf[:, ic, sl], pt[:])

    # mix^T -> combined chunks 0..3
    for ac in range(AC):
        pm = psum.tile([P, H], F32, tag="sm", name=f"pm_{h}_{ac}")
        for ic in range(IC):
            nc.tensor.matmul(
                pm[:],
                ctx_bf[:, ic, ac * P : (ac + 1) * P],
                attnT_bf[:, ic, sl],
                start=(ic == 0),
                stop=(ic == IC - 1),
            )
        nc.vector.tensor_copy(combT_bf[:, ac, sl], pm[:])


def _final_project(nc, combT_bf, out_w_bf, ones_bf, outb_row_bf, out_sb, psum,
                   out_d):
    """out = tanh(combined @ out_w + out_b) for all 64 rows at once:
    M=64 x N=512 matmuls, bias applied as a rank-1 (K=1) accumulation."""
    po = psum.tile([OUT_LEN, DEC], F32, tag="mm", name="po_final")
    for cc in range(CC):
        nc.tensor.matmul(
            po[:], combT_bf[:, cc, :], out_w_bf[:, cc, :],
            start=(cc == 0), stop=False,
        )
    nc.tensor.matmul(po[:], ones_bf[:], outb_row_bf[:], start=False, stop=True)
    nc.scalar.activation(out_sb[:], po[:], AF.Tanh)
    nc.sync.dma_start(out_d[:], out_sb[:])


def _build_body(tc):
    nc = tc.nc

    # ---- DRAM I/O (per-core shard shapes) ----
    output_d = nc.dram_tensor("output", [OUT_LEN, DEC], F32, kind="ExternalInput").ap()
    context_d = nc.dram_tensor("context", [IN_LEN, ATTN], F32, kind="ExternalInput").ap()
    dec_w_d = nc.dram_tensor("dec_w", [DEC, DEC], F32, kind="ExternalInput").ap()
    dec_b_d = nc.dram_tensor("dec_b", [DEC, 1], F32, kind="ExternalInput").ap()
    attn_w_d = nc.dram_tensor("attn_w", [ATTN, DEC], F32, kind="ExternalInput").ap()
    attn_b_d = nc.dram_tensor("attn_b", [ATTN, 1], F32, kind="ExternalInput").ap()
    query_w_d = nc.dram_tensor("query_w", [DEC, 1], F32, kind="ExternalInput").ap()
    out_w_d = nc.dram_tensor("out_w", [ATTN + DEC, DEC], F32, kind="ExternalInput").ap()
    out_b_d = nc.dram_tensor("out_b", [DEC, 1], F32, kind="ExternalInput").ap()
    out_d = nc.dram_tensor("out", [OUT_LEN, DEC], F32, kind="ExternalOutput").ap()
    attn_d = nc.dram_tensor("attn", [OUT_LEN, IN_LEN], F32, kind="ExternalOutput").ap()

    from contextlib import ExitStack

    with ExitStack() as ctx:
        const = ctx.enter_context(tc.tile_pool(name="const", bufs=1))
        statics = ctx.enter_context(tc.tile_pool(name="statics", bufs=1))
        epool = ctx.enter_context(tc.tile_pool(name="epool", bufs=4))
        fpool = ctx.enter_context(tc.tile_pool(name="fpool", bufs=3))
        spool = ctx.enter_context(tc.tile_pool(name="spool", bufs=2))
        psum = ctx.enter_context(tc.tile_pool(name="psum", bufs=2, space="PSUM"))

        # ---------------- constants / small inputs ----------------
        ident = const.tile([P, P], F32)
        make_identity(nc, ident)
        ident_bf = const.tile([P, P], BF16)
        nc.vector.tensor_copy(ident_bf[:], ident[:])

        # HAM warmup: ~4us of real matmul activity on dummy data flips the
        # PE clock gate to 8/8 (2.4 GHz) before the real matmuls arrive.
        # (PE-transpose-mode does not count as HAM activity.)
        wu = psum.tile([P, P], F32, tag="mm")
        for _ in range(16):
            nc.tensor.matmul(wu[:], ident_bf[:], ident_bf[:], start=True, stop=True)

        attn_bias = const.tile([P, DC], F32)
        dec_bias = const.tile([P, DC], F32)
        qw_f = const.tile([P, DC], F32)
        qw_bf = const.tile([P, DC], BF16)
        for tile_, dram_ in ((attn_bias, attn_b_d), (dec_bias, dec_b_d),
                             (qw_f, query_w_d)):
            nc.scalar.dma_start(
                tile_[:], dram_.rearrange("(dc p) one -> p dc one", p=P)
            )
        nc.vector.tensor_copy(qw_bf[:], qw_f[:])

        ones_bf = const.tile([1, OUT_LEN], BF16)
        nc.vector.memset(ones_bf[:], 1.0)
        outb_row_f = const.tile([1, DEC], F32)
        nc.scalar.dma_start(outb_row_f[:], out_b_d.rearrange("d one -> one d"))
        outb_row_bf = const.tile([1, DEC], BF16)
        nc.vector.tensor_copy(outb_row_bf[:], outb_row_f[:])


        # ---------------- big input DMAs (split for queue parallelism) ----
        ctx_sb = statics.tile([P, IC, ATTN], F32)      # [i%, ic, a]
        attn_w_sb = statics.tile([P, AC, DEC], F32)    # [a%, ac, d]
        dec_w_sb = statics.tile([P, EC, DEC], F32)     # [e%, ec, d]
        output_sb = statics.tile([OUT_LEN, DEC], F32)  # [o, e]
        out_w_sb = statics.tile([P, CC, DEC], F32)     # [c%, cc, d]
        ctx_bf = statics.tile([P, IC, ATTN], BF16)
        attn_w_bf = statics.tile([P, AC, DEC], BF16)
        dec_w_bf = statics.tile([P, EC, DEC], BF16)
        out_w_bf = statics.tile([P, CC, DEC], BF16)
        output_bf = statics.tile([OUT_LEN, DEC], BF16)
        for ic in range(IC):
            nc.sync.dma_start(ctx_sb[:, ic, :], context_d[ic * P : (ic + 1) * P, :])
        nc.scalar.dma_start(output_sb[:], output_d[:])
        for ac in range(AC):
            nc.scalar.dma_start(attn_w_sb[:, ac, :], attn_w_d[ac * P : (ac + 1) * P, :])
        for ic in range(IC):
            nc.vector.tensor_copy(ctx_bf[:, ic, :], ctx_sb[:, ic, :])
        for ec in range(EC):
            nc.sync.dma_start(dec_w_sb[:, ec, :], dec_w_d[ec * P : (ec + 1) * P, :])
        nc.vector.tensor_copy(output_bf[:], output_sb[:])
        for ac in range(AC):
            nc.vector.tensor_copy(attn_w_bf[:, ac, :], attn_w_sb[:, ac, :])
        for ec in range(EC):
            nc.vector.tensor_copy(dec_w_bf[:, ec, :], dec_w_sb[:, ec, :])

        # ---------------- transposes: context^T (bf16), output^T ----------
        ctxT_bf = statics.tile([P, AC, IN_LEN], BF16)  # [a%, ac, i]
        for ic in range(IC):
            for ac in range(AC):
                pt = psum.tile([P, P], BF16, tag="tp", name=f"pt_c_{ic}_{ac}")
                nc.tensor.transpose(pt[:], ctx_bf[:, ic, ac * P : (ac + 1) * P], ident_bf[:])
                nc.vector.tensor_copy(ctxT_bf[:, ac, ic * P : (ic + 1) * P], pt[:])

        # combined^T [c%, cc, o]: chunks 0..3 = mix^T (later), 4..7 = output^T
        combT_bf = statics.tile([P, CC, OUT_LEN], BF16)
        for ec in range(EC):
            pt = psum.tile([P, OUT_LEN], BF16, tag="tp", name=f"pt_ot_{ec}")
            nc.tensor.transpose(
                pt[:], output_bf[0:OUT_LEN, ec * P : (ec + 1) * P],
                ident_bf[0:OUT_LEN, 0:OUT_LEN],
            )
            nc.vector.tensor_copy(combT_bf[:, EC + ec, :], pt[:])

        # ---------------- A^T (bf16 matmul) and O^T ----------------
        ATb = statics.tile([P, DC, IN_LEN], BF16)      # [d%, dc, i]
        for dc in range(DC):
            pa = psum.tile([P, IN_LEN], F32, tag="mm", name=f"pa_{dc}")
            for ac in range(AC):
                nc.tensor.matmul(
                    pa[:],
                    attn_w_bf[:, ac, dc * P : (dc + 1) * P],
                    ctxT_bf[:, ac, :],
                    start=(ac == 0),
                    stop=(ac == AC - 1),
                )
            nc.vector.tensor_scalar_add(ATb[:, dc, :], pa[:], attn_bias[:, dc : dc + 1])

        OTb = statics.tile([P, DC, OUT_LEN], F32)      # [d%, dc, o]
        for dc in range(DC):
            po = psum.tile([P, OUT_LEN], F32, tag="sm", name=f"po_{dc}")
            for ec in range(EC):
                nc.tensor.matmul(
                    po[:],
                    dec_w_bf[:, ec, dc * P : (dc + 1) * P],
                    combT_bf[:, EC + ec, :],
                    start=(ec == 0),
                    stop=(ec == EC - 1),
                )
            nc.vector.tensor_scalar_add(OTb[:, dc, :], po[:], dec_bias[:, dc : dc + 1])

        # out_w lands during the main loop (needed first by epilogue half 0)
        for cc in range(CC):
            nc.sync.dma_start(out_w_sb[:, cc, :], out_w_d[cc * P : (cc + 1) * P, :])

        # zero-padded stationary operands: QZ[:, dc, j] is [128, G] with
        # query_w[dc] in column j, zeros elsewhere -> matmul j deposits
        # scores for o_j into PSUM row j, rows != j accumulate zeros.
        QZ = const.tile([P, DC, G, G], BF16)
        nc.vector.memset(QZ[:], 0.0)
        for dc in range(DC):
            for j in range(G):
                nc.vector.tensor_copy(QZ[:, dc, j, j : j + 1], qw_bf[:, dc : dc + 1])

        # ---------------- main loop: tanh + q-reduction ----------------
        scores_sb = statics.tile([OUT_LEN, IN_LEN], F32)
        exp_sb = statics.tile([OUT_LEN, IN_LEN], F32)
        sums = statics.tile([OUT_LEN, 1], F32)
        recip = statics.tile([OUT_LEN, 1], F32)
        attn_sb = statics.tile([OUT_LEN, IN_LEN], F32)
        attn_bf = statics.tile([OUT_LEN, IN_LEN], BF16)
        attnT_bf = statics.tile([P, IC, OUT_LEN], BF16)
        out_sb = statics.tile([OUT_LEN, DEC], F32)

        sm_args = (ident_bf, scores_sb, exp_sb, sums, recip, attn_sb, attn_bf,
                   attnT_bf, ctx_bf, combT_bf, psum, attn_d)

        for og in range(NG):
            ps8 = psum.tile([G, IN_LEN], F32, tag="sc", name=f"ps8_{og}")
            for dc in range(DC):
                E = epool.tile([P, G, IN_LEN], BF16, tag="E", name=f"E_{og}_{dc}")
                for j in range(G):
                    o = og * G + j
                    nc.vector.tensor_scalar_add(
                        E[:, j, :], ATb[:, dc, :], OTb[:, dc, o : o + 1]
                    )
                Fc = fpool.tile([P, G, IN_LEN], BF16, tag="F", name=f"F_{og}_{dc}")
                nc.scalar.activation(Fc[:], E[:], AF.Tanh)
                for j in range(G):
                    nc.tensor.matmul(
                        ps8[:],
                        QZ[:, dc, j],
                        Fc[:, j],
                        start=(dc == 0 and j == 0),
                        stop=(dc == DC - 1 and j == G - 1),
                    )
            stage8 = spool.tile([G, IN_LEN], F32, tag="st", name=f"stage8_{og}")
            nc.vector.tensor_copy(stage8[:], ps8[:])
            nc.sync.dma_start(scores_sb[og * G : (og + 1) * G, :], stage8[:])

            if og < DC:
                # out_w bf16 casts, spread over the first groups (DVE slack)
                nc.vector.tensor_copy(out_w_bf[:, 2 * og, :], out_w_sb[:, 2 * og, :])
                nc.vector.tensor_copy(
                    out_w_bf[:, 2 * og + 1, :], out_w_sb[:, 2 * og + 1, :]
                )

            if og == NG // 2:
                # rows 0..31 complete since og 3: their softmax + mix runs
                # under og 5..7 (placed here so the ACT stream never blocks)
                _epilogue_softmax_mix(nc, 0, *sm_args)

        _epilogue_softmax_mix(nc, 1, *sm_args)
        _final_project(nc, combT_bf, out_w_bf, ones_bf, outb_row_bf, out_sb,
                       psum, out_d)


_CACHE = {}


def build_nc():
    if "nc" in _CACHE:
        return _CACHE["nc"]
    nc = bacc.Bacc(
        "TRN2",
        target_bir_lowering=False,
        debug=False,
        num_devices=N_CORES,
    )
    with tile.TileContext(nc) as tc:
        _build_body(tc)
    nc.compile()
    _CACHE["nc"] = nc
    return nc


def kernel(**inputs):
    nc = build_nc()

    f = lambda k: np.ascontiguousarray(np.asarray(inputs[k], dtype=np.float32))
    output = f("output")
    context = f("context")
    shared = {
        "dec_w": f("dec_w"),
        "dec_b": f("dec_b").reshape(DEC, 1),
        "attn_w": f("attn_w"),
        "attn_b": f("attn_b").reshape(ATTN, 1),
        "query_w": f("query_w").reshape(DEC, 1),
        "out_w": f("out_w"),
        "out_b": f("out_b").reshape(DEC, 1),
    }
    in_maps = []
    for b in range(N_CORES):
        m = dict(shared)
        m["output"] = np.ascontiguousarray(output[b])
        m["context"] = np.ascontiguousarray(context[b])
        in_maps.append(m)

    res = bass_utils.run_bass_kernel_spmd(nc, in_maps, core_ids=list(range(N_CORES)))
    _CACHE["last_results"] = res
    out = np.stack([res.results[b]["out"] for b in range(N_CORES)])
    attn = np.stack([res.results[b]["attn"] for b in range(N_CORES)])
    return out, attn
